# revision 5
# baseline (speedup 1.0000x reference)
"""Trainium2 Bass kernel for nn_ArflowSparseMoeBlock (8-expert top-2 MoE, 4-layer ELU MLP).

Strategy (8 NeuronCores, expert-parallel):
  - Each core owns ONE expert's weights (w1..b4 sharded on the leading E axis).
  - hidden_states is pre-transposed on host to xT [D, T] and replicated, so the
    whole 4-layer MLP chains in feature-major layout with zero on-device
    transposes of activations.
  - Router: each core computes softmax/top-2/renorm for its own 128-token slice
    (xts input), then an AllGather shares the per-token combine weights; each
    core selects its own expert's column with a one-hot matmul.
  - Each core computes y_e = MLP_e(x) for all T=1024 tokens, scales by its
    combine column, and an AllReduce(add) produces the final output everywhere.

The kernel() entrypoint takes the FULL unsharded inputs and returns the FULL
output; sharding/replication/padding happens on host inside this file.
"""

import numpy as np

import concourse.bass as bass
import concourse.tile as tile
from concourse import bacc, mybir
from concourse.bass_utils import run_bass_kernel_spmd
from concourse.masks import make_identity

# Problem constants (hardcoded per harness rules)
D = 12336        # input features
DP = 12416       # padded to 97 * 128
KD = DP // 128   # 97 k-tiles
H = 1024         # intermediate features
O = 96           # output features
E = 8            # experts == cores
T = 1024         # tokens (B*S = 2*512)
P = 128
N_CORES = 8
TB = 512         # token block for L1-L3 matmuls (N free dim)
NB = T // TB     # 2
MG = 4           # m-group size (PSUM banks used per accumulation group)

F32 = mybir.dt.float32
BF16 = mybir.dt.bfloat16


def build(compute_dt=F32):
    """Build the SPMD Bass program (same graph for all 8 cores)."""
    nc = bacc.Bacc("TRN2", target_bir_lowering=False, debug=False,
                   num_devices=N_CORES)
    cdt = compute_dt

    # ---- I/O ----
    xt = nc.dram_tensor("xt", [DP, T], cdt, kind="ExternalInput").ap()
    xts = nc.dram_tensor("xts", [DP, P], F32, kind="ExternalInput").ap()
    gate = nc.dram_tensor("gate", [DP, E], F32, kind="ExternalInput").ap()
    w1 = nc.dram_tensor("w1", [DP, H], cdt, kind="ExternalInput").ap()
    w2 = nc.dram_tensor("w2", [H, H], cdt, kind="ExternalInput").ap()
    w3 = nc.dram_tensor("w3", [H, H], cdt, kind="ExternalInput").ap()
    w4 = nc.dram_tensor("w4", [H, O], cdt, kind="ExternalInput").ap()
    b1 = nc.dram_tensor("b1", [H, 1], F32, kind="ExternalInput").ap()
    b2 = nc.dram_tensor("b2", [H, 1], F32, kind="ExternalInput").ap()
    b3 = nc.dram_tensor("b3", [H, 1], F32, kind="ExternalInput").ap()
    b4 = nc.dram_tensor("b4", [1, O], F32, kind="ExternalInput").ap()
    oh = nc.dram_tensor("oh", [E, 1], F32, kind="ExternalInput").ap()
    out_ext = nc.dram_tensor("out", [T, O], F32, kind="ExternalOutput").ap()

    with tile.TileContext(nc) as tc:
        with (
            tc.tile_pool(name="const", bufs=1) as const,
            tc.tile_pool(name="wstream", bufs=4) as wstream,
            tc.tile_pool(name="xstream", bufs=3) as xstream,
            tc.tile_pool(name="rstream", bufs=3) as rstream,
            tc.tile_pool(name="hbuf", bufs=2) as hbuf,
            tc.tile_pool(name="small", bufs=6) as small,
            tc.tile_pool(name="epil", bufs=4) as epil,
            tc.tile_pool(name="outp", bufs=1) as outp,
            tc.tile_pool(name="psum", bufs=8, space="PSUM") as psum,
            tc.tile_pool(name="dram", bufs=1, space="DRAM") as dram,
        ):
            # ---------- constants ----------
            gate_sb = const.tile([P, KD, E], F32)
            nc.sync.dma_start(out=gate_sb,
                              in_=gate.rearrange("(k p) e -> p k e", p=P))
            w2_sb = const.tile([P, H // P, H], cdt)
            nc.sync.dma_start(out=w2_sb,
                              in_=w2.rearrange("(k p) m -> p k m", p=P))
            w3_sb = const.tile([P, H // P, H], cdt)
            nc.sync.dma_start(out=w3_sb,
                              in_=w3.rearrange("(k p) m -> p k m", p=P))
            w4_sb = const.tile([P, H // P, O], cdt)
            nc.sync.dma_start(out=w4_sb,
                              in_=w4.rearrange("(k p) o -> p k o", p=P))
            b1_sb = const.tile([P, H // P, 1], F32)
            nc.sync.dma_start(out=b1_sb,
                              in_=b1.rearrange("(m p) one -> p m one", p=P))
            b2_sb = const.tile([P, H // P, 1], F32)
            nc.sync.dma_start(out=b2_sb,
                              in_=b2.rearrange("(m p) one -> p m one", p=P))
            b3_sb = const.tile([P, H // P, 1], F32)
            nc.sync.dma_start(out=b3_sb,
                              in_=b3.rearrange("(m p) one -> p m one", p=P))
            b4_sb = const.tile([P, O], F32)
            nc.sync.dma_start(out=b4_sb, in_=b4.to_broadcast((P, O)))
            oh_sb = const.tile([E, 1], F32)
            nc.sync.dma_start(out=oh_sb, in_=oh)
            ident = const.tile([P, P], F32)
            make_identity(nc, ident)

            # ---------- router (own 128-token slice) ----------
            ps_r = psum.tile([E, P], F32, tag="sp")
            for k in range(KD):
                xk = rstream.tile([P, P], F32)
                nc.sync.dma_start(out=xk, in_=xts[k * P:(k + 1) * P, :])
                nc.tensor.matmul(ps_r, gate_sb[:, k, :], xk,
                                 start=(k == 0), stop=(k == KD - 1))
            logT = small.tile([E, P], F32)
            nc.any.tensor_copy(logT, ps_r)
            ps_t = psum.tile([P, E], F32, tag="sp")
            nc.tensor.transpose(ps_t, logT, ident[:E, :E])
            logits = small.tile([P, E], F32)
            nc.any.tensor_copy(logits, ps_t)

            mx = small.tile([P, 1], F32)
            nc.vector.reduce_max(mx, logits, axis=mybir.AxisListType.X)
            negm = small.tile([P, 1], F32)
            nc.vector.tensor_scalar_mul(negm, mx, -1.0)
            ex = small.tile([P, E], F32)
            nc.scalar.activation(ex, logits, mybir.ActivationFunctionType.Exp,
                                 bias=negm)
            sm = small.tile([P, 1], F32)
            nc.vector.reduce_sum(sm, ex, axis=mybir.AxisListType.X)
            inv = small.tile([P, 1], F32)
            nc.vector.reciprocal(inv, sm)
            prob = small.tile([P, E], F32)
            nc.vector.tensor_scalar_mul(prob, ex, inv)

            m1 = small.tile([P, 1], F32)
            nc.vector.reduce_max(m1, prob, axis=mybir.AxisListType.X)
            ismax = small.tile([P, E], F32)
            nc.vector.tensor_scalar(ismax, prob, scalar1=m1, scalar2=None,
                                    op0=mybir.AluOpType.is_ge)
            pmax = small.tile([P, E], F32)
            nc.vector.tensor_mul(pmax, prob, ismax)
            pwo = small.tile([P, E], F32)
            nc.vector.tensor_sub(pwo, prob, pmax)
            m2 = small.tile([P, 1], F32)
            nc.vector.reduce_max(m2, pwo, axis=mybir.AxisListType.X)
            ge2 = small.tile([P, E], F32)
            nc.vector.tensor_scalar(ge2, prob, scalar1=m2, scalar2=None,
                                    op0=mybir.AluOpType.is_ge)
            num = small.tile([P, E], F32)
            nc.vector.tensor_mul(num, prob, ge2)
            den = small.tile([P, 1], F32)
            nc.vector.tensor_add(den, m1, m2)
            invd = small.tile([P, 1], F32)
            nc.vector.reciprocal(invd, den)
            comb = small.tile([P, E], F32)
            nc.vector.tensor_scalar_mul(comb, num, invd)

            ps_c = psum.tile([E, P], F32, tag="sp")
            nc.tensor.transpose(ps_c, comb, ident)
            combT = small.tile([E, P], F32)
            nc.any.tensor_copy(combT, ps_c)
            combT_d = dram.tile([E, P], F32)
            nc.sync.dma_start(out=combT_d, in_=combT)
            combAll_d = dram.tile([N_CORES * E, P], F32)
            nc.gpsimd.collective_compute(
                "AllGather",
                mybir.AluOpType.bypass,
                replica_groups=[list(range(N_CORES))],
                ins=[combT_d.opt()],
                outs=[combAll_d.opt()],
            )
            combAll = const.tile([E, T // P, P], F32)
            nc.sync.dma_start(out=combAll,
                              in_=combAll_d.rearrange("(j e) t -> e j t", e=E))
            # comb column for MY expert, per 128-token block (one-hot matmul)
            comb_e = const.tile([P, T // P], F32)
            for j in range(T // P):
                ps_cj = psum.tile([P, 1], F32, tag="sp")
                nc.tensor.matmul(ps_cj, combAll[:, j, :], oh_sb,
                                 start=True, stop=True)
                nc.any.tensor_copy(comb_e[:, j:j + 1], ps_cj)

            # ---------- helpers ----------
            def elu_drain(dst, ps, bias):
                """dst = elu(ps + bias) = min(exp(x+b) - 1, relu(x+b))."""
                a = epil.tile([P, TB], F32, tag="elu_a")
                nc.scalar.activation(a, ps, mybir.ActivationFunctionType.Exp,
                                     bias=bias)
                r = epil.tile([P, TB], F32, tag="elu_r")
                nc.vector.tensor_scalar(r, ps, scalar1=bias, scalar2=0.0,
                                        op0=mybir.AluOpType.add,
                                        op1=mybir.AluOpType.max)
                nc.vector.scalar_tensor_tensor(dst, a, -1.0, r,
                                               op0=mybir.AluOpType.add,
                                               op1=mybir.AluOpType.min)

            # ---------- L1: h1 = elu(w1.T @ x + b1), feature-major ----------
            h1 = hbuf.tile([P, H // P, T], cdt, tag="h")
            for n in range(NB):
                ps = [psum.tile([P, TB], F32, tag="sp", name=f"acc{n}_{mi}")
                      for mi in range(H // P)]
                for k in range(KD):
                    w1k = wstream.tile([P, H], cdt, tag="w1k")
                    nc.sync.dma_start(out=w1k, in_=w1[k * P:(k + 1) * P, :])
                    xk = xstream.tile([P, TB], cdt, tag="xk")
                    nc.sync.dma_start(
                        out=xk,
                        in_=xt[k * P:(k + 1) * P, n * TB:(n + 1) * TB])
                    for mi in range(H // P):
                        nc.tensor.matmul(
                            ps[mi],
                            w1k[:, mi * P:(mi + 1) * P],
                            xk,
                            start=(k == 0), stop=(k == KD - 1))
                for mi in range(H // P):
                    elu_drain(h1[:, mi, n * TB:(n + 1) * TB], ps[mi],
                              b1_sb[:, mi, :])

            # ---------- L2/L3 ----------
            def mid_layer(h_in, w_sb, b_sb, lname):
                h_out = hbuf.tile([P, H // P, T], cdt, tag="h",
                                  name=f"h_{lname}")
                for n in range(NB):
                    ps = [psum.tile([P, TB], F32, tag="sp",
                                    name=f"acc_{lname}_{n}_{mi}")
                          for mi in range(H // P)]
                    for k in range(H // P):
                        for mi in range(H // P):
                            nc.tensor.matmul(
                                ps[mi],
                                w_sb[:, k, mi * P:(mi + 1) * P],
                                h_in[:, k, n * TB:(n + 1) * TB],
                                start=(k == 0), stop=(k == H // P - 1))
                    for mi in range(H // P):
                        elu_drain(h_out[:, mi, n * TB:(n + 1) * TB],
                                  ps[mi], b_sb[:, mi, :])
                return h_out

            h2 = mid_layer(h1, w2_sb, b2_sb, "l2")
            h3 = mid_layer(h2, w3_sb, b3_sb, "l3")

            # ---------- L4 + weighted combine (token-major) ----------
            out_sb = outp.tile([P, T // P, O], F32)
            for j in range(T // P):
                ps_y = psum.tile([P, O], F32, tag="sp")
                for k in range(H // P):
                    nc.tensor.matmul(ps_y,
                                     h3[:, k, j * P:(j + 1) * P],
                                     w4_sb[:, k, :],
                                     start=(k == 0), stop=(k == H // P - 1))
                t1 = epil.tile([P, O], F32, tag="l4t")
                nc.vector.tensor_add(t1, ps_y, b4_sb)
                nc.vector.tensor_scalar_mul(out_sb[:, j, :], t1,
                                            comb_e[:, j:j + 1])

            out_d = dram.tile([T, O], F32)
            nc.sync.dma_start(out=out_d.rearrange("(j p) o -> p j o", p=P),
                              in_=out_sb)
            out_red = dram.tile([T, O], F32)
            nc.gpsimd.collective_compute(
                "AllReduce",
                mybir.AluOpType.add,
                replica_groups=[list(range(N_CORES))],
                ins=[out_d.opt()],
                outs=[out_red.opt()],
            )
            nc.sync.dma_start(out=out_ext, in_=out_red)

    nc.compile()
    return nc


def _pad_rows(a, rows):
    out = np.zeros((rows,) + a.shape[1:], dtype=a.dtype)
    out[:a.shape[0]] = a
    return out


def make_in_maps(hidden_states, gate_w, w1, b1, w2, b2, w3, b3, w4, b4,
                 compute_np=np.float32):
    x = np.asarray(hidden_states, dtype=np.float32).reshape(T, D)
    xt_full = _pad_rows(np.ascontiguousarray(x.T), DP)           # [DP, T] f32
    gate_p = _pad_rows(np.asarray(gate_w, dtype=np.float32), DP)  # [DP, E]
    xt_c = xt_full.astype(compute_np)
    in_maps = []
    for i in range(N_CORES):
        ohv = np.zeros((E, 1), dtype=np.float32)
        ohv[i, 0] = 1.0
        in_maps.append({
            "xt": xt_c,
            "xts": np.ascontiguousarray(xt_full[:, i * P:(i + 1) * P]),
            "gate": gate_p,
            "w1": _pad_rows(np.asarray(w1[i], dtype=np.float32), DP).astype(compute_np),
            "w2": np.asarray(w2[i], dtype=np.float32).astype(compute_np),
            "w3": np.asarray(w3[i], dtype=np.float32).astype(compute_np),
            "w4": np.asarray(w4[i], dtype=np.float32).astype(compute_np),
            "b1": np.asarray(b1[i], dtype=np.float32).reshape(H, 1),
            "b2": np.asarray(b2[i], dtype=np.float32).reshape(H, 1),
            "b3": np.asarray(b3[i], dtype=np.float32).reshape(H, 1),
            "b4": np.asarray(b4[i], dtype=np.float32).reshape(1, O),
            "oh": ohv,
        })
    return in_maps


_NC_CACHE = {}


def get_nc(compute_dt=F32):
    key = str(compute_dt)
    if key not in _NC_CACHE:
        _NC_CACHE[key] = build(compute_dt)
    return _NC_CACHE[key]


def kernel(hidden_states, gate_w, w1, b1, w2, b2, w3, b3, w4, b4):
    import ml_dtypes
    compute_dt, compute_np = F32, np.float32
    nc = get_nc(compute_dt)
    in_maps = make_in_maps(hidden_states, gate_w, w1, b1, w2, b2, w3, b3,
                           w4, b4, compute_np=compute_np)
    res = run_bass_kernel_spmd(nc, in_maps, core_ids=list(range(N_CORES)))
    out = res.results[0]["out"]
    return np.asarray(out, dtype=np.float32).reshape(2, T // 2, O)


# revision 6
# speedup vs baseline: 1.1960x; 1.1960x over previous
"""Trainium2 Bass kernel for nn_ArflowSparseMoeBlock (8-expert top-2 MoE, 4-layer ELU MLP).

Strategy (8 NeuronCores, expert-parallel):
  - Each core owns ONE expert's weights (w1..b4 sharded on the leading E axis).
  - hidden_states is pre-transposed (and padded D 12336->12416) on host to
    xT [D, T] and replicated, so the whole 4-layer MLP chains in feature-major
    layout with zero on-device transposes of activations.
  - Router: each core computes softmax/top-2/renorm for its own 128-token
    slice (xts input), an AllGather shares the per-token combine weights, and
    each core selects its own expert's column with a one-hot matmul.
  - Each core computes y_e = MLP_e(x) for all T=1024 tokens (bf16 matmuls,
    fp32 accumulation), scales by its combine column, and an AllReduce(add)
    produces the final output everywhere.
  - All constant/streamed tensors are pre-arranged on host into
    partition-major layouts so every DMA is large and contiguous per
    partition.

The kernel() entrypoint takes the FULL unsharded inputs and returns the FULL
output; sharding/replication/padding happens on host inside this file.
"""

import numpy as np

import concourse.bass as bass
import concourse.tile as tile
from concourse import bacc, mybir
from concourse.bass_utils import run_bass_kernel_spmd
from concourse.masks import make_identity

# Problem constants (hardcoded per harness rules)
D = 12336        # input features
DP = 12416       # padded to 97 * 128
P = 128
KD = DP // P     # 97 k-tiles
H = 1024         # intermediate features
O = 96           # output features
E = 8            # experts == cores
T = 1024         # tokens (B*S = 2*512)
N_CORES = 8
TB = 512         # token block (matmul moving free dim)
NB = T // TB     # 2
KG = 4           # k-tiles per streamed DMA (1MB w1 chunks)
KGS = [(g * KG, min(KG, KD - g * KG)) for g in range((KD + KG - 1) // KG)]
RCH = 25         # router k-tiles per xts chunk
MT = H // P      # 8 m-tiles

F32 = mybir.dt.float32
BF16 = mybir.dt.bfloat16


def build(compute_dt=BF16):
    """Build the SPMD Bass program (identical graph on all 8 cores)."""
    nc = bacc.Bacc("TRN2", target_bir_lowering=False, debug=False,
                   num_devices=N_CORES)
    cdt = compute_dt

    # ---- I/O (all pre-arranged on host, partition-major) ----
    xt = nc.dram_tensor("xt", [P, NB, KD, TB], cdt, kind="ExternalInput").ap()
    xts = nc.dram_tensor("xts", [P, KD, P], F32, kind="ExternalInput").ap()
    gate = nc.dram_tensor("gate", [P, KD, E], F32, kind="ExternalInput").ap()
    w1 = nc.dram_tensor("w1", [P, KD, H], cdt, kind="ExternalInput").ap()
    w2 = nc.dram_tensor("w2", [P, MT, H], cdt, kind="ExternalInput").ap()
    w3 = nc.dram_tensor("w3", [P, MT, H], cdt, kind="ExternalInput").ap()
    w4 = nc.dram_tensor("w4", [P, MT, O], cdt, kind="ExternalInput").ap()
    b1 = nc.dram_tensor("b1", [P, MT], F32, kind="ExternalInput").ap()
    b2 = nc.dram_tensor("b2", [P, MT], F32, kind="ExternalInput").ap()
    b3 = nc.dram_tensor("b3", [P, MT], F32, kind="ExternalInput").ap()
    b4 = nc.dram_tensor("b4", [1, O], F32, kind="ExternalInput").ap()
    oh = nc.dram_tensor("oh", [E, 1], F32, kind="ExternalInput").ap()
    out_ext = nc.dram_tensor("out", [T, O], F32, kind="ExternalOutput").ap()

    with tile.TileContext(nc) as tc:
        with (
            tc.tile_pool(name="const", bufs=1) as const,
            tc.tile_pool(name="wstream", bufs=3) as wstream,
            tc.tile_pool(name="xstream", bufs=3) as xstream,
            tc.tile_pool(name="rstream", bufs=2) as rstream,
            tc.tile_pool(name="hbuf", bufs=2) as hbuf,
            tc.tile_pool(name="small", bufs=6) as small,
            tc.tile_pool(name="epil", bufs=3) as epil,
            tc.tile_pool(name="outp", bufs=1) as outp,
            tc.tile_pool(name="psum", bufs=8, space="PSUM") as psum,
            tc.tile_pool(name="dram", bufs=1, space="DRAM") as dram,
        ):
            # ---------- early constants (router needs these) ----------
            gate_sb = const.tile([P, KD, E], F32)
            nc.sync.dma_start(out=gate_sb, in_=gate)
            b1_sb = const.tile([P, MT], F32)
            nc.sync.dma_start(out=b1_sb, in_=b1)
            ident = const.tile([P, P], F32)
            make_identity(nc, ident)

            # ---------- router (own 128-token slice) ----------
            ps_r = psum.tile([E, P], F32, tag="sp")
            for ci, (c0, cn) in enumerate(
                    [(i * RCH, min(RCH, KD - i * RCH))
                     for i in range((KD + RCH - 1) // RCH)]):
                xc = rstream.tile([P, RCH, P], F32, tag="xc", name=f"xc{ci}")
                nc.sync.dma_start(out=xc[:, :cn, :], in_=xts[:, c0:c0 + cn, :])
                for k in range(cn):
                    nc.tensor.matmul(ps_r, gate_sb[:, c0 + k, :], xc[:, k, :],
                                     start=(c0 + k == 0),
                                     stop=(c0 + k == KD - 1))
            logT = small.tile([E, P], F32)
            nc.any.tensor_copy(logT, ps_r)
            ps_t = psum.tile([P, E], F32, tag="sp")
            nc.tensor.transpose(ps_t, logT, ident[:E, :E])
            logits = small.tile([P, E], F32)
            nc.any.tensor_copy(logits, ps_t)

            mx = small.tile([P, 1], F32)
            nc.vector.reduce_max(mx, logits, axis=mybir.AxisListType.X)
            negm = small.tile([P, 1], F32)
            nc.vector.tensor_scalar_mul(negm, mx, -1.0)
            ex = small.tile([P, E], F32)
            nc.scalar.activation(ex, logits, mybir.ActivationFunctionType.Exp,
                                 bias=negm)
            sm = small.tile([P, 1], F32)
            nc.vector.reduce_sum(sm, ex, axis=mybir.AxisListType.X)
            inv = small.tile([P, 1], F32)
            nc.vector.reciprocal(inv, sm)
            prob = small.tile([P, E], F32)
            nc.vector.tensor_scalar_mul(prob, ex, inv)

            m1 = small.tile([P, 1], F32)
            nc.vector.reduce_max(m1, prob, axis=mybir.AxisListType.X)
            ismax = small.tile([P, E], F32)
            nc.vector.tensor_scalar(ismax, prob, scalar1=m1, scalar2=None,
                                    op0=mybir.AluOpType.is_ge)
            pmax = small.tile([P, E], F32)
            nc.vector.tensor_mul(pmax, prob, ismax)
            pwo = small.tile([P, E], F32)
            nc.vector.tensor_sub(pwo, prob, pmax)
            m2 = small.tile([P, 1], F32)
            nc.vector.reduce_max(m2, pwo, axis=mybir.AxisListType.X)
            ge2 = small.tile([P, E], F32)
            nc.vector.tensor_scalar(ge2, prob, scalar1=m2, scalar2=None,
                                    op0=mybir.AluOpType.is_ge)
            num = small.tile([P, E], F32)
            nc.vector.tensor_mul(num, prob, ge2)
            den = small.tile([P, 1], F32)
            nc.vector.tensor_add(den, m1, m2)
            invd = small.tile([P, 1], F32)
            nc.vector.reciprocal(invd, den)
            comb = small.tile([P, E], F32)
            nc.vector.tensor_scalar_mul(comb, num, invd)

            ps_c = psum.tile([E, P], F32, tag="sp")
            nc.tensor.transpose(ps_c, comb, ident)
            combT = small.tile([E, P], F32)
            nc.any.tensor_copy(combT, ps_c)
            combT_d = dram.tile([E, P], F32)
            nc.sync.dma_start(out=combT_d, in_=combT)
            combAll_d = dram.tile([N_CORES * E, P], F32)
            nc.gpsimd.collective_compute(
                "AllGather",
                mybir.AluOpType.bypass,
                replica_groups=[list(range(N_CORES))],
                ins=[combT_d.opt()],
                outs=[combAll_d.opt()],
            )

            # ---------- helpers ----------
            def elu_drain(dst, ps, bias):
                """dst = elu(ps + bias) = min(exp(x+b) - 1, relu(x+b))."""
                a = epil.tile([P, TB], F32, tag="elu_a")
                nc.scalar.activation(a, ps, mybir.ActivationFunctionType.Exp,
                                     bias=bias)
                r = epil.tile([P, TB], F32, tag="elu_r")
                nc.vector.tensor_scalar(r, ps, scalar1=bias, scalar2=0.0,
                                        op0=mybir.AluOpType.add,
                                        op1=mybir.AluOpType.max)
                nc.vector.scalar_tensor_tensor(dst, a, -1.0, r,
                                               op0=mybir.AluOpType.add,
                                               op1=mybir.AluOpType.min)

            # ---------- L1: h1 = elu(w1.T @ x + b1), feature-major ----------
            h1 = hbuf.tile([P, MT, T], cdt, tag="h", name="h_l1")
            for n in range(NB):
                ps = [psum.tile([P, TB], F32, tag="sp", name=f"acc1_{n}_{mi}")
                      for mi in range(MT)]
                for gi, (k0, kn) in enumerate(KGS):
                    w1g = wstream.tile([P, KG, H], cdt, tag="w1g",
                                       name=f"w1g_{n}_{gi}")
                    nc.sync.dma_start(out=w1g[:, :kn, :],
                                      in_=w1[:, k0:k0 + kn, :])
                    xg = xstream.tile([P, KG, TB], cdt, tag="xg",
                                      name=f"xg_{n}_{gi}")
                    nc.sync.dma_start(out=xg[:, :kn, :],
                                      in_=xt[:, n, k0:k0 + kn, :])
                    for k in range(kn):
                        for mi in range(MT):
                            nc.tensor.matmul(
                                ps[mi],
                                w1g[:, k, mi * P:(mi + 1) * P],
                                xg[:, k, :],
                                start=(k0 + k == 0),
                                stop=(k0 + k == KD - 1))
                for mi in range(MT):
                    elu_drain(h1[:, mi, n * TB:(n + 1) * TB], ps[mi],
                              b1_sb[:, mi:mi + 1])

            # ---------- late constants (overlap their DMA with L1) ----------
            w2_sb = const.tile([P, MT, H], cdt)
            nc.sync.dma_start(out=w2_sb, in_=w2)
            b2_sb = const.tile([P, MT], F32)
            nc.sync.dma_start(out=b2_sb, in_=b2)
            w3_sb = const.tile([P, MT, H], cdt)
            nc.sync.dma_start(out=w3_sb, in_=w3)
            b3_sb = const.tile([P, MT], F32)
            nc.sync.dma_start(out=b3_sb, in_=b3)
            w4_sb = const.tile([P, MT, O], cdt)
            nc.sync.dma_start(out=w4_sb, in_=w4)
            b4_sb = const.tile([P, O], F32)
            nc.sync.dma_start(out=b4_sb, in_=b4.to_broadcast((P, O)))
            oh_sb = const.tile([E, 1], F32)
            nc.sync.dma_start(out=oh_sb, in_=oh)

            # ---------- L2/L3 ----------
            def mid_layer(h_in, w_sb, b_sb, lname):
                h_out = hbuf.tile([P, MT, T], cdt, tag="h",
                                  name=f"h_{lname}")
                for n in range(NB):
                    ps = [psum.tile([P, TB], F32, tag="sp",
                                    name=f"acc_{lname}_{n}_{mi}")
                          for mi in range(MT)]
                    for k in range(MT):
                        for mi in range(MT):
                            nc.tensor.matmul(
                                ps[mi],
                                w_sb[:, k, mi * P:(mi + 1) * P],
                                h_in[:, k, n * TB:(n + 1) * TB],
                                start=(k == 0), stop=(k == MT - 1))
                    for mi in range(MT):
                        elu_drain(h_out[:, mi, n * TB:(n + 1) * TB],
                                  ps[mi], b_sb[:, mi:mi + 1])
                return h_out

            h2 = mid_layer(h1, w2_sb, b2_sb, "l2")
            h3 = mid_layer(h2, w3_sb, b3_sb, "l3")

            # ---------- comb column select (AllGather result, one-hot mm) ----
            combAll = const.tile([E, T // P, P], F32)
            nc.sync.dma_start(out=combAll,
                              in_=combAll_d.rearrange("(j e) t -> e j t", e=E))
            comb_e = const.tile([P, T // P], F32)
            for j in range(T // P):
                ps_cj = psum.tile([P, 1], F32, tag="sp")
                nc.tensor.matmul(ps_cj, combAll[:, j, :], oh_sb,
                                 start=True, stop=True)
                nc.any.tensor_copy(comb_e[:, j:j + 1], ps_cj)

            # ---------- L4 + weighted combine (token-major) ----------
            out_sb = outp.tile([P, T // P, O], F32)
            for j in range(T // P):
                ps_y = psum.tile([P, O], F32, tag="sp")
                for k in range(MT):
                    nc.tensor.matmul(ps_y,
                                     h3[:, k, j * P:(j + 1) * P],
                                     w4_sb[:, k, :],
                                     start=(k == 0), stop=(k == MT - 1))
                t1 = epil.tile([P, O], F32, tag="l4t")
                nc.vector.tensor_add(t1, ps_y, b4_sb)
                nc.vector.tensor_scalar_mul(out_sb[:, j, :], t1,
                                            comb_e[:, j:j + 1])

            out_d = dram.tile([T, O], F32)
            nc.sync.dma_start(out=out_d.rearrange("(j p) o -> p j o", p=P),
                              in_=out_sb)
            out_red = dram.tile([T, O], F32)
            nc.gpsimd.collective_compute(
                "AllReduce",
                mybir.AluOpType.add,
                replica_groups=[list(range(N_CORES))],
                ins=[out_d.opt()],
                outs=[out_red.opt()],
            )
            nc.sync.dma_start(out=out_ext, in_=out_red)

    nc.compile()
    return nc


def _pad_rows(a, rows):
    out = np.zeros((rows,) + a.shape[1:], dtype=a.dtype)
    out[:a.shape[0]] = a
    return out


def _pkm(a, dt):
    """[K*P, M] row-major -> [P, K, M] partition-major, cast to dt."""
    kp, m = a.shape
    return np.ascontiguousarray(
        a.reshape(kp // P, P, m).transpose(1, 0, 2)).astype(dt)


def make_in_maps(hidden_states, gate_w, w1, b1, w2, b2, w3, b3, w4, b4,
                 compute_np=None):
    if compute_np is None:
        import ml_dtypes
        compute_np = ml_dtypes.bfloat16
    x = np.asarray(hidden_states, dtype=np.float32).reshape(T, D)
    xt_full = _pad_rows(np.ascontiguousarray(x.T), DP)            # [DP, T] f32
    # xt: [P, NB, KD, TB]
    xt_r = np.ascontiguousarray(
        xt_full.reshape(KD, P, NB, TB).transpose(1, 2, 0, 3)).astype(compute_np)
    gate_r = _pkm(_pad_rows(np.asarray(gate_w, dtype=np.float32), DP),
                  np.float32)                                      # [P, KD, E]
    in_maps = []
    for i in range(N_CORES):
        ohv = np.zeros((E, 1), dtype=np.float32)
        ohv[i, 0] = 1.0
        xts_r = np.ascontiguousarray(
            xt_full[:, i * P:(i + 1) * P].reshape(KD, P, P).transpose(1, 0, 2))
        in_maps.append({
            "xt": xt_r,
            "xts": xts_r,
            "gate": gate_r,
            "w1": _pkm(_pad_rows(np.asarray(w1[i], dtype=np.float32), DP),
                       compute_np),
            "w2": _pkm(np.asarray(w2[i], dtype=np.float32), compute_np),
            "w3": _pkm(np.asarray(w3[i], dtype=np.float32), compute_np),
            "w4": _pkm(np.asarray(w4[i], dtype=np.float32), compute_np),
            "b1": np.ascontiguousarray(
                np.asarray(b1[i], dtype=np.float32).reshape(MT, P).T),
            "b2": np.ascontiguousarray(
                np.asarray(b2[i], dtype=np.float32).reshape(MT, P).T),
            "b3": np.ascontiguousarray(
                np.asarray(b3[i], dtype=np.float32).reshape(MT, P).T),
            "b4": np.asarray(b4[i], dtype=np.float32).reshape(1, O),
            "oh": ohv,
        })
    return in_maps


_NC_CACHE = {}


def get_nc(compute_dt=BF16):
    key = str(compute_dt)
    if key not in _NC_CACHE:
        _NC_CACHE[key] = build(compute_dt)
    return _NC_CACHE[key]


def kernel(hidden_states, gate_w, w1, b1, w2, b2, w3, b3, w4, b4):
    nc = get_nc(BF16)
    in_maps = make_in_maps(hidden_states, gate_w, w1, b1, w2, b2, w3, b3,
                           w4, b4)
    res = run_bass_kernel_spmd(nc, in_maps, core_ids=list(range(N_CORES)))
    out = res.results[0]["out"]
    return np.asarray(out, dtype=np.float32).reshape(2, T // 2, O)
